# revision 16
# baseline (speedup 1.0000x reference)
"""Causal self-attention (B=2, S=2048, D=2048, H=16) on 8 TRN2 NeuronCores.

Sharding: 2 batches x 4 head-groups.  Core c handles batch c//4 and heads
[4*(c%4) .. 4*(c%4)+3]; each core produces output columns [512*(c%4) ...].

Schedule (~400-430us traced vs the 464us baseline; PE busy ~357us is the
floor under the board's ~74% GPIO power throttle):
- Ascending q-blocks with per-token-block head-0 projections: attention
  starts ~14us in and AG(0,0) triggers ~55us in, giving every collective
  large slack.  Diagonal k-tile matmuls are narrowed to the causally
  valid q-range.
- GpSimd carries ONLY the AG triggers + one-time setup: instructions on
  that queue after a collective trigger block until the collective
  completes (the startup barrier alone wedges it 45-100us).  The softmax
  denominator broadcast is an all-ones [128,128] stationary matmul
  (column sums replicated across partitions, same 512-row PE cost as a
  [1,512] ones-matmul) + full-width DVE reciprocal -- no gpsimd, no PE
  bubble.
- Each dma_start costs ~600ns on the issuing engine's queue, so DMA
  count and queue placement matter: xt/wqk1-3/wout/ygt/ag_in strips on
  sync, wv/wqk0/out strips on scalar; xt tokens 512:2048 load as
  [128,1536] row tiles.
- AllGathers become consumable at trigger + peer-launch-skew (~10-30us,
  varies) + transfer (~11-15us).  Head 3's second half gathers per
  q-block (small last AG), and all three remaining out-projection passes
  run after attention ends: ~51us of AG-independent PE work that hides
  the last AGs' latency deterministically.  ygt tiles prefetch 2.5
  passes deep so the passes stay matmul-paced.

Softmax uses exp without max subtraction (logits are O(8) here); denominators
are accumulated on DVE over k-tile pairs then reduced across partitions with
a single ones-matmul, inverted with reciprocal_approx_fast.

Compute is bf16 with fp32 PSUM accumulation.
"""

import numpy as np
import ml_dtypes

B, S, D = 2, 2048, 2048
H, HD = 16, 128
HLOC = 4           # heads per core
CW = HLOC * HD     # 512: per-core v width and out-column width
QB = 4             # q blocks of 512
DT = 16            # d tiles of 128
TB = 4             # token blocks of 512
SCALE = 1.0 / float(np.sqrt(HD))
GROUPS = [[0, 1, 2, 3], [4, 5, 6, 7]]

_cache = {}


def _build():
    import concourse.tile as tile
    import concourse.mybir as mybir
    from concourse import bacc

    BF = mybir.dt.bfloat16
    F32 = mybir.dt.float32

    nc = bacc.Bacc("TRN2", target_bir_lowering=False, debug=False, num_devices=8)

    # Inputs (per-core shards, host-prepared)
    xT = nc.dram_tensor("xT", [D, S], BF, kind="ExternalInput")          # x[batch].T
    # wqk[h][qk] partition-major: [128 d-within-dt, DT*128 qcols-by-dt]
    wqk = nc.dram_tensor("wqk", [HLOC, 2, 128, D], BF, kind="ExternalInput")
    wv = nc.dram_tensor("wv", [DT, 128, CW], BF, kind="ExternalInput")
    bqk = nc.dram_tensor("bqk", [128, HLOC * 2], F32, kind="ExternalInput")
    bv = nc.dram_tensor("bv", [1, CW], F32, kind="ExternalInput")
    # w_out rows permuted: wout[h][i] = w_out[512*i + 128*h : +128, cols]
    wout = nc.dram_tensor("wout", [HLOC, 4, 128, CW], BF, kind="ExternalInput")
    bout = nc.dram_tensor("bout", [1, CW], F32, kind="ExternalInput")
    out = nc.dram_tensor("out", [S, CW], F32, kind="ExternalOutput")

    # AG buffers per (head, part). Parts: 0 = tokens 0:1024 (qb0+qb1) for all
    # heads; 1 = tokens 1024:2048 for h<3; head 3 splits its second half into
    # part 2 (tokens 1024:1536) and part 3 (1536:2048) so the last AG is small.
    def ag_parts(h):
        return (0, 1) if h < HLOC - 1 else (0, 2, 3)

    def ag_width(part):
        return 1024 if part in (0, 1) else 512

    ag_in = {}
    ag_out = {}
    for h in range(HLOC):
        for part in ag_parts(h):
            w = ag_width(part)
            ag_in[(h, part)] = nc.dram_tensor(
                f"ag_in{h}_{part}", [128, w], BF, kind="Internal")
            ag_out[(h, part)] = nc.dram_tensor(
                f"ag_out{h}_{part}", [512, w], BF, kind="Internal")

    with tile.TileContext(nc) as tc:
        with (
            tc.tile_pool(name="const", bufs=1) as constp,
            tc.tile_pool(name="pers", bufs=1) as pers,
            tc.tile_pool(name="work", bufs=2) as work,
            tc.tile_pool(name="psum", bufs=2, space="PSUM") as psum,
        ):
            # ---- constants ----
            ones128 = constp.tile([128, 128], BF, name="ones128")
            nc.gpsimd.memset(ones128[:], 1.0)

            # Pair masks for the 4 diagonal k-subtiles, packed two subtiles
            # wide: pairmask[m][:, 512*sub + qq] keeps where
            # qq >= kk + 128*(2m+sub).
            pairmasks = []
            for pm in range(2):
                m = constp.tile([128, 1024], BF, name=f"pmask{pm}",
                                tag=f"pmask{pm}")
                nc.gpsimd.memset(m[:], 1.0)
                for sub in range(2):
                    nc.gpsimd.affine_select(
                        out=m[:, sub * 512:(sub + 1) * 512],
                        in_=m[:, sub * 512:(sub + 1) * 512],
                        compare_op=mybir.AluOpType.is_ge, fill=0.0,
                        base=-128 * (2 * pm + sub), channel_multiplier=-1,
                        pattern=[[1, 512]],
                    )
                pairmasks.append(m)

            bout_sb = constp.tile([1, CW], F32, name="bout_sb")
            nc.sync.dma_start(bout_sb[:], bout[:])
            bias_bc = constp.tile([128, CW], F32, name="bias_bc")
            nc.gpsimd.partition_broadcast(bias_bc[:], bout_sb[:], channels=128)

            bv_sb = constp.tile([1, CW], F32, name="bv_sb")
            nc.sync.dma_start(bv_sb[:], bv[:])
            vbias_bc = constp.tile([128, CW], F32, name="vbias_bc")
            nc.gpsimd.partition_broadcast(vbias_bc[:], bv_sb[:], channels=128)

            bqk_sb = constp.tile([128, HLOC * 2], F32, name="bqk_sb")
            nc.sync.dma_start(bqk_sb[:], bqk[:])

            def bqk_ap(h, qk):
                return bqk_sb[:, 2 * h + qk:2 * h + qk + 1]

            # ---- persistent tiles ----
            vt = [pers.tile([128, CW], BF, name=f"v{t}", tag=f"v{t}")
                  for t in range(16)]

            # ---- loads, ordered by first use, issue spread over engines ----
            # xt0[dt] = [128,512] tokens 0:512; xtr[dt] = [128,1536] rest
            xt0 = {}
            xtr = {}
            wv_sb = []
            for dt in range(DT):
                wvp = work.tile([128, CW], BF, name=f"wvp{dt}", tag="wv",
                                bufs=16)
                nc.scalar.dma_start(wvp[:], wv[dt])
                wv_sb.append(wvp)
                t = work.tile([128, 512], BF, name=f"xt0_{dt}", tag="xT0",
                              bufs=16)
                nc.sync.dma_start(t[:], xT[dt * 128:(dt + 1) * 128, 0:512])
                xt0[dt] = t

            w_sb = {}

            def load_wqk(h, eng):
                for qk in range(2):
                    t = work.tile([128, D], BF, name=f"w_{h}_{qk}", tag="w",
                                  bufs=4)
                    for half in range(2):
                        eng.dma_start(
                            t[:, half * 1024:(half + 1) * 1024],
                            wqk[h, qk][:, half * 1024:(half + 1) * 1024])
                    w_sb[(h, qk)] = t

            load_wqk(0, nc.scalar)
            for dt in range(DT):
                t = work.tile([128, 1536], BF, name=f"xtr_{dt}", tag="xTr",
                              bufs=16)
                nc.sync.dma_start(t[:], xT[dt * 128:(dt + 1) * 128, 512:2048])
                xtr[dt] = t
            load_wqk(1, nc.sync)
            # wout as 8 merged [128,1024] tiles: wout2[(h,ip)][:, k*512:...]
            # holds wout[h, 2*ip+k]
            wout2 = {}
            for h in range(HLOC):
                for ip in range(2):
                    t = work.tile([128, 1024], BF, name=f"wout{h}{ip}",
                                  tag="wout", bufs=8)
                    for k in range(2):
                        nc.sync.dma_start(t[:, k * 512:(k + 1) * 512],
                                            wout[h, 2 * ip + k])
                    wout2[(h, ip)] = t
            load_wqk(2, nc.sync)
            load_wqk(3, nc.sync)

            def wout_ap(h, i):
                return wout2[(h, i // 2)][:, (i % 2) * 512:(i % 2 + 1) * 512]

            def xt_ap(dt, tb):
                if tb == 0:
                    return xt0[dt][:]
                return xtr[dt][:, (tb - 1) * 512:tb * 512]

            # ---- v projection for one token block: v[4tb..4tb+3] ----
            def v_tb(tb):
                for j in range(4):
                    t = 4 * tb + j
                    acc = psum.tile([128, CW], F32, name="acc_v", tag="acc",
                                    bufs=2)
                    for dt in range(DT):
                        xs = xt_ap(dt, tb)
                        nc.tensor.matmul(
                            acc[:],
                            xs[:, j * 128:(j + 1) * 128],
                            wv_sb[dt][:],
                            start=(dt == 0), stop=(dt == DT - 1),
                        )
                    nc.vector.tensor_tensor(vt[t][:], acc[:], vbias_bc[:],
                                            mybir.AluOpType.add)

            # ---- q/k projection for one (head, qk, token block) ----
            qkT = {}

            def qk_tb(h, qk, tb):
                if (h, qk) not in qkT:
                    qkT[(h, qk)] = work.tile([128, S], BF, name=f"qkT_{h}_{qk}",
                                             tag="qkT", bufs=4)
                dest = qkT[(h, qk)]
                acc = psum.tile([128, 512], F32, name="acc_qk", tag="acc",
                                bufs=2)
                for dt in range(DT):
                    nc.tensor.matmul(
                        acc[:],
                        w_sb[(h, qk)][:, dt * 128:(dt + 1) * 128],
                        xt_ap(dt, tb),
                        start=(dt == 0), stop=(dt == DT - 1),
                    )
                nc.scalar.activation(
                    dest[:, tb * 512:(tb + 1) * 512], acc[:],
                    mybir.ActivationFunctionType.Identity,
                    bias=bqk_ap(h, qk), scale=1.0,
                )

            def qk_proj(h):
                for qk in range(2):
                    for tb in range(TB):
                        qk_tb(h, qk, tb)

            # ---- attention for one (head, q-block), ascending qb ----
            # k-tile order: the 4 diagonal tiles first (causally narrowed),
            # then the full tiles; the last av is forced full-width so the
            # PSUM accumulation group closes over the whole region.
            def attn_qb(h, qb):
                qTh, kTh = qkT[(h, 0)], qkT[(h, 1)]
                nk = 4 * qb + 4
                kts = list(range(4 * qb, nk)) + list(range(0, 4 * qb))
                pairs = [(kts[2 * i], kts[2 * i + 1]) for i in range(nk // 2)]
                first_use = (h == 0 and qb == 0)  # PSUM may hold non-finite

                y_ps = psum.tile([128, 512], F32, name="y_ps", tag="y")
                esum = work.tile([128, 1024], BF, name="esum", tag="esum",
                                 bufs=2)

                def qlo(kt):
                    return 128 * (kt - 4 * qb) if kt >= 4 * qb else 0

                def flush(prev_pair, last):
                    e, pr = prev_pair
                    for s_ in range(2):
                        kt = pairs[pr][s_]
                        lo = 0 if (last and s_ == 1) else qlo(kt)
                        nc.tensor.matmul(
                            y_ps[:, lo:512],
                            vt[kt][:, h * 128:(h + 1) * 128],
                            e[:, s_ * 512 + lo:(s_ + 1) * 512],
                            start=(pr == 0 and s_ == 0),
                            stop=(last and s_ == 1),
                        )
                    if pr == 0:
                        nc.vector.tensor_copy(esum[:], e[:])
                    else:
                        nc.vector.tensor_tensor(esum[:], esum[:], e[:],
                                                mybir.AluOpType.add)

                prev = None
                for pr in range(nk // 2):
                    sc = psum.tile([128, 1024], F32, name="sc", tag="s",
                                   bufs=2)
                    for s_ in range(2):
                        kt = pairs[pr][s_]
                        lo = 0 if first_use else qlo(kt)
                        nc.tensor.matmul(
                            sc[:, s_ * 512 + lo:(s_ + 1) * 512],
                            kTh[:, kt * 128:(kt + 1) * 128],
                            qTh[:, qb * 512 + lo:(qb + 1) * 512],
                            start=True, stop=True,
                        )
                    e = work.tile([128, 1024], BF, name="expT", tag="expT",
                                  bufs=4)
                    nc.scalar.activation(
                        e[:], sc[:], mybir.ActivationFunctionType.Exp,
                        scale=SCALE,
                    )
                    if pr < 2:
                        nc.vector.tensor_tensor(e[:], e[:], pairmasks[pr][:],
                                                mybir.AluOpType.mult)
                    if prev is not None:
                        flush(prev, last=False)
                    prev = (e, pr)
                flush(prev, last=True)

                esum_f = work.tile([128, 512], BF, name="esum_f",
                                   tag="esum_f", bufs=1)
                nc.vector.tensor_tensor(esum_f[:], esum[:, 0:512],
                                        esum[:, 512:1024],
                                        mybir.AluOpType.add)
                sum_bc = psum.tile([128, 512], F32, name="sum_bc", tag="y")
                nc.tensor.matmul(sum_bc[:], ones128[:], esum_f[:],
                                 start=True, stop=True)
                rbc = work.tile([128, 512], F32, name="rbc", tag="rbc",
                                bufs=2)
                nc.vector.reciprocal_approx_fast(rbc[:], sum_bc[:])
                ynorm = work.tile([128, 512], BF, name="ynorm", tag="ynorm",
                                  bufs=2)
                nc.vector.tensor_tensor(ynorm[:], y_ps[:], rbc[:],
                                        mybir.AluOpType.mult)

                # store into the AG input buffer (vector queue); the last
                # head's small parts go as two strips for lower latency
                if qb < 2:
                    part, co = 0, (qb % 2) * 512
                elif h < HLOC - 1:
                    part, co = 1, (qb % 2) * 512
                else:
                    part, co = qb, 0
                for st in range(2):
                    nc.sync.dma_start(
                        ag_in[(h, part)][:, co + st * 256:co + (st + 1) * 256],
                        ynorm[:, st * 256:(st + 1) * 256])
                if qb == 1 or (qb == 3 and h < HLOC - 1) or \
                        (h == HLOC - 1 and qb in (2, 3)):
                    nc.gpsimd.collective_compute(
                        "AllGather", mybir.AluOpType.bypass,
                        replica_groups=GROUPS,
                        ins=[ag_in[(h, part)].ap()],
                        outs=[ag_out[(h, part)].ap()],
                    )

            def attn_head(h):
                for qb in range(QB):
                    attn_qb(h, qb)

            # ---- out-projection partial pass for head-chunk h ----
            part_acc = {}
            ygt_pre = {}

            def load_ygt(h, tc_):
                if tc_ < 2:
                    src, co = ag_out[(h, 0)], (tc_ % 2) * 512
                elif h < HLOC - 1:
                    src, co = ag_out[(h, 1)], (tc_ % 2) * 512
                else:
                    src, co = ag_out[(h, tc_)], 0
                tiles = []
                for i in range(4):
                    t = work.tile([128, 512], BF, name=f"yg_{h}_{tc_}_{i}",
                                  tag="ygt", bufs=10)
                    if h == HLOC - 1:
                        for st, eng in ((0, nc.sync), (1, nc.scalar)):
                            eng.dma_start(
                                t[:, st * 256:(st + 1) * 256],
                                src[i * 128:(i + 1) * 128,
                                    co + st * 256:co + (st + 1) * 256])
                    else:
                        nc.sync.dma_start(
                            t[:], src[i * 128:(i + 1) * 128, co:co + 512])
                    tiles.append(t)
                return tiles

            def prefetch_ygt(h, tcs):
                for tc_ in tcs:
                    ygt_pre[(h, tc_)] = load_ygt(h, tc_)

            def outproj_pass(h):
                for tc_ in range(4):
                    ygt = ygt_pre.pop((h, tc_), None) or load_ygt(h, tc_)
                    for j in range(4):
                        t = tc_ * 4 + j
                        acc = psum.tile([128, CW], F32, name="acc_o",
                                        tag="acc", bufs=2)
                        for i in range(4):
                            nc.tensor.matmul(
                                acc[:],
                                ygt[i][:, j * 128:(j + 1) * 128],
                                wout_ap(h, i),
                                start=(i == 0), stop=(i == 3),
                            )
                        if h == 0:
                            p = work.tile([128, CW], BF, name=f"part{t}",
                                          tag=f"part{t}", bufs=1)
                            part_acc[t] = p
                            nc.vector.tensor_tensor(p[:], acc[:], bias_bc[:],
                                                    mybir.AluOpType.add)
                        elif h < HLOC - 1:
                            nc.vector.tensor_tensor(part_acc[t][:],
                                                    part_acc[t][:], acc[:],
                                                    mybir.AluOpType.add)
                        else:
                            osb = work.tile([128, CW], F32, name="osb",
                                            tag="osb", bufs=2)
                            nc.vector.tensor_tensor(osb[:], part_acc[t][:],
                                                    acc[:],
                                                    mybir.AluOpType.add)
                            for st, eng in ((0, nc.sync), (1, nc.scalar)):
                                eng.dma_start(
                                    out[t * 128:(t + 1) * 128,
                                        st * 256:(st + 1) * 256],
                                    osb[:, st * 256:(st + 1) * 256])

            # ---- schedule ----
            # Head 0 incrementally per token block so attention starts early
            # and AG(0,0) triggers ~50us in; subsequent heads pipelined with
            # the next head's projection and an out-projection pass placed
            # where its AG has had cover to land.  Only pass(3) is exposed.
            for tb in range(TB):
                v_tb(tb)
                qk_tb(0, 0, tb)
                qk_tb(0, 1, tb)
                attn_qb(0, tb)
            qk_proj(1)
            attn_head(1)
            qk_proj(2)
            outproj_pass(0)
            attn_head(2)
            qk_proj(3)
            prefetch_ygt(1, (0, 1))
            attn_head(3)
            outproj_pass(1)
            outproj_pass(2)
            outproj_pass(3)

    nc.compile()
    return nc


def _prep_inputs(x, w_qkv, b_qkv, w_out, b_out):
    """Host-side sharding/layout. Returns in_maps for the 8 cores."""
    bf16 = ml_dtypes.bfloat16
    x = np.asarray(x, dtype=np.float32)
    w_qkv = np.asarray(w_qkv, dtype=np.float32)
    b_qkv = np.asarray(b_qkv, dtype=np.float32)
    w_out = np.asarray(w_out, dtype=np.float32)
    b_out = np.asarray(b_out, dtype=np.float32)

    xT_b = [np.ascontiguousarray(x[b].T).astype(bf16) for b in range(B)]

    in_maps = []
    for c in range(8):
        b, g = c // 4, c % 4
        cols = slice(CW * g, CW * (g + 1))

        # wqk[h][qk] partition-major [128, 2048]: row p holds, for each dt,
        # w_qkv[dt*128+p, cols of head]
        wqk = np.empty((HLOC, 2, 128, D), np.float32)
        bqk = np.empty((128, HLOC * 2), np.float32)
        for h in range(HLOC):
            gh = 4 * g + h
            for qk in range(2):
                wcol = w_qkv[:, qk * D + 128 * gh: qk * D + 128 * (gh + 1)]
                wqk[h, qk] = wcol.reshape(DT, 128, 128).transpose(1, 0, 2) \
                                 .reshape(128, D)
                bqk[:, 2 * h + qk] = b_qkv[qk * D + 128 * gh: qk * D + 128 * (gh + 1)]

        wv_ = w_qkv[:, 2 * D:3 * D][:, cols]
        bv_ = b_qkv[2 * D:3 * D][cols]

        # w_out rows permuted to the AG's rank-major order per head chunk
        wout_loc = w_out[:, cols]
        wout_t = np.empty((HLOC, 4, 128, CW), np.float32)
        for h in range(HLOC):
            for i in range(4):
                wout_t[h, i] = wout_loc[512 * i + 128 * h: 512 * i + 128 * (h + 1), :]

        in_maps.append({
            "xT": xT_b[b],
            "wqk": np.ascontiguousarray(wqk).astype(bf16),
            "wv": np.ascontiguousarray(wv_.reshape(DT, 128, CW)).astype(bf16),
            "bqk": np.ascontiguousarray(bqk),
            "bv": np.ascontiguousarray(bv_.reshape(1, CW)),
            "wout": np.ascontiguousarray(wout_t).astype(bf16),
            "bout": np.ascontiguousarray(b_out[cols].reshape(1, CW)),
        })
    return in_maps


def kernel(x, w_qkv, b_qkv, w_out, b_out, _trace=False, _trace_kwargs=None):
    from concourse.bass_utils import run_bass_kernel_spmd

    if "nc" not in _cache:
        _cache["nc"] = _build()
    nc = _cache["nc"]

    in_maps = _prep_inputs(x, w_qkv, b_qkv, w_out, b_out)
    res = run_bass_kernel_spmd(
        nc, in_maps, core_ids=list(range(8)),
        trace=_trace, **(_trace_kwargs or {}),
    )

    out = np.empty((B, S, D), dtype=np.float32)
    for c in range(8):
        b, g = c // 4, c % 4
        out[b][:, CW * g:CW * (g + 1)] = res.results[c]["out"]
    kernel.last_result = res
    return out
